# revision 23
# baseline (speedup 1.0000x reference)
"""GNN encoder (Linear+ReLU -> mean-aggregation SAGEConv) on 8 TRN2 NeuronCores.

Self-contained: hardcodes problem shapes (N=100000, XD=512, HID=64, E=1e6).

v3 design (measured: SWDGE gather ~54GB/s floor; collectives and gathers
contend on SDMA, so keep ONE AllGather and shorten everything else):
  - Nodes sharded across 8 cores (12500 each, padded to 12544 = 98 tiles).
  - Phase 1: xT pre-transposed on HOST (bf16 [512, SHP]) -> contiguous DMA,
    pipelined with PE; hT kept SBUF-resident as hT_aug [65, SHP] bf16
    (row 64 = ones, folds the bl bias into the Wr matmul).
  - Node-major h (bf16, 256B-stride rows, garbage pad never read) -> ag_in;
    single AllGather -> 100352-row table per core.
  - Phase 2 (v1 block schedule): edges grouped by (dst tile of 128, src bank
    of 25088 rows), chunked by 128 (shared max-over-cores schedule);
    dma_gather 8 chunks/instr (NI=1024, hard ucode max) on 4 SWDGE queues;
    per chunk one matmul psum[64,128] += msg.T @ B with B a PURE 0/1
    one-hot in fp8 (exact); psum accumulates across all 4 banks per tile
    within a block of 7 tiles.
  - Combine per tile at block end: ps_a = sumT.T @ WlT scaled by exact f32
    inv-degree (per-partition tensor_scalar); ps_b = hT_aug.T @ [WrT; bl];
    out = ps_a*inv + ps_b.
"""

import numpy as np
import ml_dtypes

N_NODES = 100000
XD = 512
HID = 64
N_CORES = 8
SH = N_NODES // N_CORES          # 12500
P = 128
T_TILES = 98                     # ceil(12500/128)
SHP = T_TILES * P                # 12544
NTAB = SHP * N_CORES             # 100352
N_BANKS = 4
BANK = NTAB // N_BANKS           # 25088
BLOCK_TILES = 6
MAX_CHUNKS_PER_INSTR = 8         # NI = 1024 (hard ucode limit)

TRACE = False
LAST_EXEC_NS = None
LAST_RES = None


def _prep(edge_index):
    """Host-side sharding/scheduling. Returns shared schedule + per-core data."""
    src = np.asarray(edge_index[0], dtype=np.int64)
    dst = np.asarray(edge_index[1], dtype=np.int64)

    # degree-balanced permutation per core: sort local nodes by in-degree,
    # deal round-robin across tiles -> every (tile,bank) group ~equal load.
    # pos[c][old_local_row] = new local row (node-major layout is permuted
    # consistently: xT columns, table rows, out rows).
    pos = np.zeros((N_CORES, SH), dtype=np.int64)
    for c in range(N_CORES):
        sel = (dst >= c * SH) & (dst < (c + 1) * SH)
        degc = np.bincount(dst[sel] - c * SH, minlength=SH)
        order_n = np.argsort(-degc, kind="stable")
        newrow = np.empty(SH, dtype=np.int64)
        idx = np.arange(SH)
        newrow[order_n] = (idx % T_TILES) * P + idx // T_TILES
        pos[c] = newrow

    per_core = []
    counts_all = np.zeros((N_CORES, T_TILES * N_BANKS), dtype=np.int64)
    for c in range(N_CORES):
        sel = (dst >= c * SH) & (dst < (c + 1) * SH)
        e_src = src[sel]
        e_ld = pos[c][(dst[sel] - c * SH).astype(np.int64)]
        deg = np.bincount(e_ld, minlength=SHP)
        inv = (1.0 / np.maximum(deg, 1)).astype(np.float32)
        sc = e_src // SH
        r_new = pos[sc, e_src % SH]
        half = r_new // (SHP // 2)
        row_in_half = sc * (SHP // 2) + (r_new % (SHP // 2))
        bank = half * 2 + row_in_half // BANK
        blocal = (row_in_half % BANK).astype(np.int64)
        tt = e_ld // P
        key = tt * N_BANKS + bank
        order = np.argsort(key, kind="stable")
        per_core.append({
            "key": key[order],
            "blocal": blocal[order].astype(np.int32),
            "dstloc": (e_ld[order] % P).astype(np.int32),
            "inv_tile": inv[:SHP].reshape(T_TILES, P),
        })
        counts_all[c] = np.bincount(key, minlength=T_TILES * N_BANKS)

    # shared chunk counts per (tile, bank): max over cores
    q_tb = -(-counts_all.max(axis=0) // P).reshape(T_TILES, N_BANKS)

    # chunk schedule: per block of tiles, bank-major for long same-bank runs
    sched_t, sched_b = [], []
    blocks = []
    block_tiles_list = []
    b0 = 0
    for sz in [2] * 9:
        block_tiles_list.append(list(range(b0, b0 + sz)))
        b0 += sz
    while b0 < T_TILES:
        sz = min(BLOCK_TILES, T_TILES - b0)
        block_tiles_list.append(list(range(b0, b0 + sz)))
        b0 += sz
    NPRE = 9  # first NPRE blocks: banks 0-1 for all of them first, then 2-3
    def emit_block_banks(tiles, bank_list):
        for b in bank_list:
            for t in tiles:
                for _ in range(q_tb[t, b]):
                    sched_t.append(t)
                    sched_b.append(b)
    for blk in range(NPRE):
        emit_block_banks(block_tiles_list[blk], [0, 1])
    for blk in range(NPRE):
        emit_block_banks(block_tiles_list[blk], [2, 3])
        blocks.append((block_tiles_list[blk], None, len(sched_t)))
    for blk in range(NPRE, len(block_tiles_list)):
        emit_block_banks(block_tiles_list[blk], [0, 1, 2, 3])
        blocks.append((block_tiles_list[blk], None, len(sched_t)))
    blocks.sort(key=lambda x: x[2])
    sched_t = np.array(sched_t, dtype=np.int64)
    sched_b = np.array(sched_b, dtype=np.int64)
    nch = len(sched_t)

    # instruction list: batch consecutive same-bank chunks (<= 8)
    instrs = []  # (chunk_start, n_chunks, bank)
    i = 0
    while i < nch:
        j = i
        while j < nch and j - i < MAX_CHUNKS_PER_INSTR and sched_b[j] == sched_b[i]:
            j += 1
        instrs.append((i, j - i, int(sched_b[i])))
        i = j

    # part 0 = prefix-tile banks 0-1; part 1 = everything else
    NPRE_TILES = 18
    def part_of(ci):
        return 0 if (sched_t[ci] < NPRE_TILES and sched_b[ci] < 2) else 1
    first_ch = np.full((2, T_TILES), -1, dtype=np.int64)
    last_ch = np.full((2, T_TILES), -1, dtype=np.int64)
    for ci in range(nch):
        t = sched_t[ci]
        pp = part_of(ci)
        if first_ch[pp, t] < 0:
            first_ch[pp, t] = ci
        last_ch[pp, t] = ci

    # chunk slot offset within its (t,b) group
    grp_seen = {}
    chunk_q = np.zeros(nch, dtype=np.int64)
    for ci in range(nch):
        k = (int(sched_t[ci]), int(sched_b[ci]))
        chunk_q[ci] = grp_seen.get(k, 0)
        grp_seen[k] = chunk_q[ci] + 1

    # per-core fill of gather idx / one-hot B (pure 0/1 fp8)
    core_arrays = []
    for c in range(N_CORES):
        pc = per_core[c]
        cnts = counts_all[c]
        starts = np.zeros(T_TILES * N_BANKS + 1, dtype=np.int64)
        np.cumsum(cnts, out=starts[1:])
        gidx = np.zeros((nch, P), dtype=np.int16)
        dstloc = np.full((nch, P), 255, dtype=np.int64)
        for ci in range(nch):
            t, b, q = int(sched_t[ci]), int(sched_b[ci]), int(chunk_q[ci])
            g = t * N_BANKS + b
            s = starts[g] + q * P
            n = min(P, starts[g + 1] - s)
            if n <= 0:
                continue
            sl = slice(s, s + n)
            gidx[ci, :n] = pc["blocal"][sl]
            dstloc[ci, :n] = pc["dstloc"][sl]
        onehot = (dstloc[:, :, None] == np.arange(P)[None, None, :])
        bbig = onehot.astype(ml_dtypes.float8_e4m3fn)
        bbig = np.ascontiguousarray(bbig.transpose(1, 0, 2).reshape(P, nch * P))
        idx16 = gidx.reshape(nch, 8, 16).transpose(2, 0, 1).reshape(16, nch * 8)
        idx128 = np.tile(idx16, (8, 1))
        core_arrays.append({
            "gidx": np.ascontiguousarray(idx128),
            "bbig": bbig,
            "invt": np.ascontiguousarray(pc["inv_tile"].T),  # [P, T_TILES]
        })

    meta = {
        "pos": pos,
        "nch": nch,
        "instrs": instrs,
        "sched_t": sched_t,
        "first_ch": first_ch,
        "last_ch": last_ch,
        "blocks": blocks,
        "has_chunks": (q_tb.sum(axis=1) > 0),
        "hasP": (q_tb[:, :2].sum(axis=1) > 0),
        "hasR": (q_tb[:, 2:].sum(axis=1) > 0),
        "npre_tiles": NPRE_TILES,
    }
    return meta, core_arrays


_GATHER_PATCHED = False


def _relax_gather_elem_assert():
    """dma_gather asserts elem_size_bytes % 256 == 0 (a transpose-mode
    restriction applied unconditionally). The non-transpose ucode handles
    128-byte payloads with a 256-byte row stride (verified on hardware).
    Rebuild the method with the assert relaxed to % 128."""
    global _GATHER_PATCHED
    if _GATHER_PATCHED:
        return
    import inspect
    import re
    import concourse.bass as bassmod

    src = inspect.getsource(bassmod.BassGpSimd.dma_gather)
    src = src.replace(
        "elem_size_bytes > 0 and elem_size_bytes % 256 == 0",
        "elem_size_bytes > 0 and elem_size_bytes % 128 == 0",
    )
    src = re.sub(r"^    def ", "def ", src, count=1, flags=re.M)
    src = "\n".join(l[4:] if l.startswith("    ") else l for l in src.split("\n"))
    ns = dict(bassmod.__dict__)
    exec(compile(src, "patched_dma_gather", "exec"), ns)
    bassmod.BassGpSimd.dma_gather = ns["dma_gather"]
    _GATHER_PATCHED = True


def _build_program(meta):
    import concourse.bass as bass
    import concourse.bacc as bacc
    import concourse.mybir as mybir
    import concourse.tile as tile

    _relax_gather_elem_assert()

    nch = meta["nch"]
    gcols = nch * 8

    nc = bacc.Bacc("TRN2", target_bir_lowering=False, debug=False,
                   num_devices=N_CORES, num_swdge_queues=4)
    f32 = mybir.dt.float32
    bf16 = mybir.dt.bfloat16
    fp8 = mybir.dt.float8e4

    xT_in = nc.dram_tensor("xT", [XD, SHP], bf16, kind="ExternalInput")
    w1t = nc.dram_tensor("w1t", [XD, HID], bf16, kind="ExternalInput")
    b1 = nc.dram_tensor("b1", [HID, 1], f32, kind="ExternalInput")
    wlt = nc.dram_tensor("wlt", [HID, HID], bf16, kind="ExternalInput")
    wra = nc.dram_tensor("wra", [HID + 1, HID], bf16, kind="ExternalInput")
    ident_in = nc.dram_tensor("ident", [P, P], bf16, kind="ExternalInput")
    bbig_in = nc.dram_tensor("bbig", [P, nch * P], fp8, kind="ExternalInput")
    gidx_in = nc.dram_tensor("gidx", [P, gcols], mybir.dt.int16, kind="ExternalInput")
    invt_in = nc.dram_tensor("invt", [P, T_TILES], f32, kind="ExternalInput")

    out_d = nc.dram_tensor("out", [SHP, HID], f32, kind="ExternalOutput")

    HSH = SHP // 2                    # 6272 rows per half per core
    ag_in = [nc.dram_tensor(f"ag_in{h}", [HSH, 2 * HID], bf16)
             for h in range(2)]
    ag_out = [nc.dram_tensor(f"ag_out{h}", [HSH * N_CORES, 2 * HID], bf16,
                             addr_space="Shared")
              for h in range(2)]
    ones_in = nc.dram_tensor("ones", [1, SHP], bf16, kind="ExternalInput")

    with tile.TileContext(nc) as tc:
        with (
            tc.tile_pool(name="const", bufs=1) as cpool,
            tc.tile_pool(name="idx", bufs=1) as ipool,
        ):
            w1t_sb = cpool.tile([P, 4, HID], bf16)
            nc.sync.dma_start(
                out=w1t_sb[:],
                in_=w1t.ap().rearrange("(k p) d -> p k d", p=P),
            )
            b1_sb = cpool.tile([HID, 1], f32)
            nc.sync.dma_start(out=b1_sb[:], in_=b1[:])
            wlt_sb = cpool.tile([HID, HID], bf16)
            nc.scalar.dma_start(out=wlt_sb[:], in_=wlt[:])
            wra_sb = cpool.tile([HID + 1, HID], bf16)
            nc.scalar.dma_start(out=wra_sb[:], in_=wra[:])
            ident_sb = cpool.tile([P, P], bf16)
            nc.scalar.dma_start(out=ident_sb[:], in_=ident_in[:])
            invt_sb = cpool.tile([P, T_TILES], f32)
            nc.scalar.dma_start(out=invt_sb[:], in_=invt_in[:])
            gidx_sb = ipool.tile([P, gcols], mybir.dt.int16)
            nc.scalar.dma_start(out=gidx_sb[:], in_=gidx_in[:])

            # hT_aug [65, SHP] bf16, row 64 = ones (bias fold)
            hT_sb = cpool.tile([HID + 1, SHP], bf16)
            nc.scalar.dma_start(out=hT_sb[HID : HID + 1, :], in_=ones_in[:])

            # ---------------- Phase 1: hT = relu(W1 @ xT + b1) -------------
            with (
                tc.tile_pool(name="xT", bufs=4) as xpool,
                tc.tile_pool(name="p1ps", bufs=4, space="PSUM") as p1ps,
                tc.tile_pool(name="p1tr", bufs=4, space="PSUM") as p1tr,
                tc.tile_pool(name="p1h", bufs=8) as p1h,
            ):
                def emit_transposes(g0, gw, h):
                    ns = gw // P
                    hrow = p1h.tile([P, 4, HID], bf16, tag="hrow", name="hrow")
                    for s in range(ns):
                        tp = p1tr.tile([P, HID], bf16, tag="tp", name="tp",
                                       space="PSUM")
                        nc.tensor.transpose(
                            out=tp[:],
                            in_=hT_sb[:HID, g0 + s * P : g0 + (s + 1) * P],
                            identity=ident_sb[:HID, :HID],
                        )
                        nc.vector.tensor_copy(out=hrow[:, s, :], in_=tp[:])
                    r0 = g0 - h * HSH
                    nc.scalar.dma_start(
                        out=ag_in[h].ap()[r0 : r0 + gw, :HID].rearrange(
                            "(s p) d -> p s d", p=P),
                        in_=hrow[:, :ns, :],
                    )

                XB = 1536
                for h in range(2):
                    pending = None
                    xt = None
                    for g0 in range(h * HSH, (h + 1) * HSH, 512):
                        gw = min(512, (h + 1) * HSH - g0)
                        off = (g0 - h * HSH) % XB
                        if off == 0:
                            xw = min(XB, (h + 1) * HSH - g0)
                            xt = xpool.tile([P, 4, XB], bf16, tag="xt")
                            nc.sync.dma_start(
                                out=xt[:, :, :xw],
                                in_=xT_in.ap()[:, g0 : g0 + xw].rearrange(
                                    "(k p) n -> p k n", p=P),
                            )
                        hps = p1ps.tile([HID, 512], f32, tag="hps", space="PSUM")
                        for k in range(4):
                            nc.tensor.matmul(
                                out=hps[:, :gw],
                                lhsT=w1t_sb[:, k, :],
                                rhs=xt[:, k, off : off + gw],
                                start=(k == 0),
                                stop=(k == 3),
                            )
                        nc.scalar.activation(
                            out=hT_sb[:HID, g0 : g0 + gw], in_=hps[:, :gw],
                            func=mybir.ActivationFunctionType.Relu,
                            bias=b1_sb[:], scale=1.0,
                        )
                        if pending is not None:
                            emit_transposes(*pending)
                        pending = (g0, gw, h)
                    emit_transposes(*pending)
                    # half complete -> fire its AllGather
                    nc.gpsimd.collective_compute(
                        "AllGather",
                        mybir.AluOpType.bypass,
                        replica_groups=[list(range(N_CORES))],
                        ins=[ag_in[h].ap().opt()],
                        outs=[ag_out[h].ap().opt()],
                    )

            # ---------------- Phase 2: gather + aggregate + combine --------
            instrs = meta["instrs"]
            sched_t = meta["sched_t"]
            first_ch = meta["first_ch"]
            last_ch = meta["last_ch"]
            blocks = meta["blocks"]
            has_chunks = meta["has_chunks"]
            hasP = meta["hasP"]
            hasR = meta["hasR"]
            NPRE_TILES = meta["npre_tiles"]
            acc_sb = cpool.tile([HID, NPRE_TILES, P], f32, name="acc_sb")

            with (
                tc.tile_pool(name="msgbf", bufs=24) as mbfpool,
                tc.tile_pool(name="bmat", bufs=24) as bpool,
                tc.tile_pool(name="agg", bufs=1, space="PSUM") as apool,
                tc.tile_pool(name="cps", bufs=1, space="PSUM") as cpspool,
                tc.tile_pool(name="comb", bufs=8) as combpool,
            ):
                ptiles = {}
                blk_done = [False] * len(blocks)

                qn = 0
                for ii, (c0, nch_i, bank) in enumerate(instrs):
                    ni = nch_i * P
                    msgbf = mbfpool.tile([P, MAX_CHUNKS_PER_INSTR * HID], bf16,
                                         tag="msgbf")
                    nc.gpsimd.dma_gather(
                        msgbf[:, : nch_i * HID].rearrange("p (c d) -> p c d", d=HID),
                        ag_out[bank // 2][(bank % 2) * BANK : (bank % 2 + 1) * BANK, :HID],
                        gidx_sb[:, c0 * 8 : c0 * 8 + nch_i * 8],
                        ni, ni, HID,
                        elem_step=2 * HID,
                        queue_num=qn,
                    )
                    qn = (qn + 1) % 4
                    btile = bpool.tile([P, MAX_CHUNKS_PER_INSTR * P], fp8, tag="bt")
                    nc.sync.dma_start(
                        out=btile[:, : nch_i * P],
                        in_=bbig_in[:, c0 * P : (c0 + nch_i) * P],
                    )
                    for k in range(nch_i):
                        ci = c0 + k
                        t = int(sched_t[ci])
                        pp = 0 if (t < NPRE_TILES and bank < 2) else 1
                        if t not in ptiles:
                            ptiles[t] = apool.tile(
                                [HID, P], f32, tag=f"agg{t % 6}",
                                name="aggtile", space="PSUM"
                            )
                        nc.tensor.matmul(
                            out=ptiles[t][:],
                            lhsT=msgbf[:, k * HID : (k + 1) * HID],
                            rhs=btile[:, k * P : (k + 1) * P],
                            start=(ci == first_ch[pp][t]),
                            stop=(ci == last_ch[pp][t]),
                        )
                        if pp == 0 and ci == last_ch[0][t]:
                            nc.vector.tensor_copy(
                                out=acc_sb[:, t, :], in_=ptiles.pop(t)[:])
                    # blocks fully processed? emit their combines
                    nxt = instrs[ii + 1][0] if ii + 1 < len(instrs) else meta["nch"]
                    for blk, (tiles, _, ce) in enumerate(blocks):
                        if blk_done[blk] or nxt < ce:
                            continue
                        blk_done[blk] = True
                        for t in tiles:
                            if t < NPRE_TILES:
                                psum_part = (ptiles.pop(t)
                                             if hasR[t] else None)
                                _combine(nc, mybir, combpool, cpspool,
                                         hT_sb, wlt_sb, wra_sb, invt_sb,
                                         out_d, t, psum_part,
                                         acc_sb if hasP[t] else None)
                            else:
                                psum_part = (ptiles.pop(t)
                                             if has_chunks[t] else None)
                                _combine(nc, mybir, combpool, cpspool,
                                         hT_sb, wlt_sb, wra_sb, invt_sb,
                                         out_d, t, psum_part, None)

    nc.compile()
    return nc


def _combine(nc, mybir, combpool, cpspool, hT_sb, wlt_sb, wra_sb,
             invt_sb, out_d, t, psum_part, acc_sb):
    f32 = mybir.dt.float32
    bf16 = mybir.dt.bfloat16
    cps_b = cpspool.tile([P, HID], f32, tag="cpsb", name="cpsb", space="PSUM")
    nc.tensor.matmul(
        out=cps_b[:],
        lhsT=hT_sb[:, t * P : (t + 1) * P],
        rhs=wra_sb[:],
        start=True, stop=True,
    )
    if psum_part is not None or acc_sb is not None:
        sumT = combpool.tile([HID, P], bf16, tag="sumT", name="sumT")
        if psum_part is not None and acc_sb is not None:
            nc.vector.tensor_tensor(
                out=sumT[:], in0=psum_part[:], in1=acc_sb[:, t, :],
                op=mybir.AluOpType.add,
            )
        elif psum_part is not None:
            nc.vector.tensor_copy(out=sumT[:], in_=psum_part[:])
        else:
            nc.vector.tensor_copy(out=sumT[:], in_=acc_sb[:, t, :])
        cps_a = cpspool.tile([P, HID], f32, tag="cpsa", name="cpsa", space="PSUM")
        nc.tensor.matmul(
            out=cps_a[:], lhsT=sumT[:], rhs=wlt_sb[:],
            start=True, stop=True,
        )
        scaled = combpool.tile([P, HID], f32, tag="scaled", name="scaled")
        nc.vector.tensor_scalar_mul(
            scaled[:], cps_a[:], invt_sb[:, t : t + 1],
        )
        out_sb = combpool.tile([P, HID], f32, tag="outsb", name="outsb")
        nc.vector.tensor_tensor(
            out=out_sb[:], in0=scaled[:], in1=cps_b[:],
            op=mybir.AluOpType.add,
        )
    else:
        out_sb = combpool.tile([P, HID], f32, tag="outsb", name="outsb")
        nc.vector.tensor_copy(out=out_sb[:], in_=cps_b[:])
    nc.scalar.dma_start(out=out_d[t * P : (t + 1) * P, :], in_=out_sb[:])


def kernel(x, edge_index, W1, b1, Wl, bl, Wr):
    from concourse.bass_utils import run_bass_kernel_spmd

    x = np.asarray(x)
    edge_index = np.asarray(edge_index)
    W1 = np.asarray(W1, dtype=np.float32)
    b1v = np.asarray(b1, dtype=np.float32)
    Wl = np.asarray(Wl, dtype=np.float32)
    blv = np.asarray(bl, dtype=np.float32)
    Wr = np.asarray(Wr, dtype=np.float32)

    meta, core_arrays = _prep(edge_index)
    nc = _build_program(meta)

    # host-side transpose + pad + bf16
    xT_np = np.zeros((N_CORES, XD, SHP), dtype=ml_dtypes.bfloat16)
    xb = x.astype(ml_dtypes.bfloat16)
    pos = meta["pos"]
    for c in range(N_CORES):
        xT_np[c, :, pos[c]] = xb[c * SH : (c + 1) * SH]
    w1t_np = np.ascontiguousarray(W1.T).astype(ml_dtypes.bfloat16)
    b1_np = np.ascontiguousarray(b1v[:, None])
    wlt_np = np.ascontiguousarray(Wl.T).astype(ml_dtypes.bfloat16)
    wra_np = np.concatenate([Wr.T, blv[None, :]], axis=0).astype(
        ml_dtypes.bfloat16)
    ident_np = np.eye(P, dtype=np.float32).astype(ml_dtypes.bfloat16)
    ones_np = np.ones((1, SHP), dtype=ml_dtypes.bfloat16)

    in_maps = []
    for c in range(N_CORES):
        ca = core_arrays[c]
        in_maps.append({
            "xT": np.ascontiguousarray(xT_np[c]),
            "w1t": w1t_np,
            "b1": b1_np,
            "wlt": wlt_np,
            "wra": wra_np,
            "ident": ident_np,
            "ones": ones_np,
            "bbig": ca["bbig"],
            "gidx": ca["gidx"],
            "invt": ca["invt"],
        })

    global LAST_EXEC_NS, LAST_RES
    res = run_bass_kernel_spmd(nc, in_maps, list(range(N_CORES)), trace=TRACE)
    LAST_EXEC_NS = res.exec_time_ns
    LAST_RES = res
    out = np.empty((N_NODES, HID), dtype=np.float32)
    for c in range(N_CORES):
        out[c * SH : (c + 1) * SH] = res.results[c]["out"][meta["pos"][c]]
    return out


# revision 24
# speedup vs baseline: 1.0354x; 1.0354x over previous
"""GNN encoder (Linear+ReLU -> mean-aggregation SAGEConv) on 8 TRN2 NeuronCores.

Self-contained: hardcodes problem shapes (N=100000, XD=512, HID=64, E=1e6).

v3 design (measured: SWDGE gather ~54GB/s floor; collectives and gathers
contend on SDMA, so keep ONE AllGather and shorten everything else):
  - Nodes sharded across 8 cores (12500 each, padded to 12544 = 98 tiles).
  - Phase 1: xT pre-transposed on HOST (bf16 [512, SHP]) -> contiguous DMA,
    pipelined with PE; hT kept SBUF-resident as hT_aug [65, SHP] bf16
    (row 64 = ones, folds the bl bias into the Wr matmul).
  - Node-major h (bf16, 256B-stride rows, garbage pad never read) -> ag_in;
    single AllGather -> 100352-row table per core.
  - Phase 2 (v1 block schedule): edges grouped by (dst tile of 128, src bank
    of 25088 rows), chunked by 128 (shared max-over-cores schedule);
    dma_gather 8 chunks/instr (NI=1024, hard ucode max) on 4 SWDGE queues;
    per chunk one matmul psum[64,128] += msg.T @ B with B a PURE 0/1
    one-hot in fp8 (exact); psum accumulates across all 4 banks per tile
    within a block of 7 tiles.
  - Combine per tile at block end: ps_a = sumT.T @ WlT scaled by exact f32
    inv-degree (per-partition tensor_scalar); ps_b = hT_aug.T @ [WrT; bl];
    out = ps_a*inv + ps_b.
"""

import numpy as np
import ml_dtypes

N_NODES = 100000
XD = 512
HID = 64
N_CORES = 8
SH = N_NODES // N_CORES          # 12500
P = 128
T_TILES = 98                     # ceil(12500/128)
SHP = T_TILES * P                # 12544
NTAB = SHP * N_CORES             # 100352
N_BANKS = 4
BANK = NTAB // N_BANKS           # 25088
BLOCK_TILES = 6
MAX_CHUNKS_PER_INSTR = 8         # NI = 1024 (hard ucode limit)

TRACE = False
LAST_EXEC_NS = None
LAST_RES = None


def _prep(edge_index):
    """Host-side sharding/scheduling. Returns shared schedule + per-core data."""
    src = np.asarray(edge_index[0], dtype=np.int64)
    dst = np.asarray(edge_index[1], dtype=np.int64)

    # degree-balanced permutation per core: sort local nodes by in-degree,
    # deal round-robin across tiles -> every (tile,bank) group ~equal load.
    # pos[c][old_local_row] = new local row (node-major layout is permuted
    # consistently: xT columns, table rows, out rows).
    pos = np.zeros((N_CORES, SH), dtype=np.int64)
    for c in range(N_CORES):
        sel = (dst >= c * SH) & (dst < (c + 1) * SH)
        degc = np.bincount(dst[sel] - c * SH, minlength=SH)
        order_n = np.argsort(-degc, kind="stable")
        newrow = np.empty(SH, dtype=np.int64)
        idx = np.arange(SH)
        newrow[order_n] = (idx % T_TILES) * P + idx // T_TILES
        pos[c] = newrow

    per_core = []
    counts_all = np.zeros((N_CORES, T_TILES * N_BANKS), dtype=np.int64)
    for c in range(N_CORES):
        sel = (dst >= c * SH) & (dst < (c + 1) * SH)
        e_src = src[sel]
        e_ld = pos[c][(dst[sel] - c * SH).astype(np.int64)]
        deg = np.bincount(e_ld, minlength=SHP)
        inv = (1.0 / np.maximum(deg, 1)).astype(np.float32)
        sc = e_src // SH
        r_new = pos[sc, e_src % SH]
        half = r_new // (SHP // 2)
        row_in_half = sc * (SHP // 2) + (r_new % (SHP // 2))
        bank = half * 2 + row_in_half // BANK
        blocal = (row_in_half % BANK).astype(np.int64)
        tt = e_ld // P
        key = tt * N_BANKS + bank
        order = np.argsort(key, kind="stable")
        per_core.append({
            "key": key[order],
            "blocal": blocal[order].astype(np.int32),
            "dstloc": (e_ld[order] % P).astype(np.int32),
            "inv_tile": inv[:SHP].reshape(T_TILES, P),
        })
        counts_all[c] = np.bincount(key, minlength=T_TILES * N_BANKS)

    # shared chunk counts per (tile, bank): max over cores
    q_tb = -(-counts_all.max(axis=0) // P).reshape(T_TILES, N_BANKS)

    # chunk schedule: per block of tiles, bank-major for long same-bank runs
    sched_t, sched_b = [], []
    blocks = []
    block_tiles_list = []
    b0 = 0
    for sz in [2, 2, 2, 2, 2, 2]:
        block_tiles_list.append(list(range(b0, b0 + sz)))
        b0 += sz
    while b0 < T_TILES:
        sz = min(BLOCK_TILES, T_TILES - b0)
        block_tiles_list.append(list(range(b0, b0 + sz)))
        b0 += sz
    NPRE = 6  # first NPRE blocks: banks 0-1 for all of them first, then 2-3
    def emit_block_banks(tiles, bank_list):
        for b in bank_list:
            for t in tiles:
                for _ in range(q_tb[t, b]):
                    sched_t.append(t)
                    sched_b.append(b)
    for blk in range(NPRE):
        emit_block_banks(block_tiles_list[blk], [0, 1])
    for blk in range(NPRE):
        emit_block_banks(block_tiles_list[blk], [2, 3])
        blocks.append((block_tiles_list[blk], None, len(sched_t)))
    for blk in range(NPRE, len(block_tiles_list)):
        emit_block_banks(block_tiles_list[blk], [0, 1, 2, 3])
        blocks.append((block_tiles_list[blk], None, len(sched_t)))
    blocks.sort(key=lambda x: x[2])
    sched_t = np.array(sched_t, dtype=np.int64)
    sched_b = np.array(sched_b, dtype=np.int64)
    nch = len(sched_t)

    # instruction list: batch consecutive same-bank chunks (<= 8)
    instrs = []  # (chunk_start, n_chunks, bank)
    i = 0
    while i < nch:
        j = i
        while j < nch and j - i < MAX_CHUNKS_PER_INSTR and sched_b[j] == sched_b[i]:
            j += 1
        instrs.append((i, j - i, int(sched_b[i])))
        i = j

    # part 0 = prefix-tile banks 0-1; part 1 = everything else
    NPRE_TILES = 12
    def part_of(ci):
        return 0 if (sched_t[ci] < NPRE_TILES and sched_b[ci] < 2) else 1
    first_ch = np.full((2, T_TILES), -1, dtype=np.int64)
    last_ch = np.full((2, T_TILES), -1, dtype=np.int64)
    for ci in range(nch):
        t = sched_t[ci]
        pp = part_of(ci)
        if first_ch[pp, t] < 0:
            first_ch[pp, t] = ci
        last_ch[pp, t] = ci

    # chunk slot offset within its (t,b) group
    grp_seen = {}
    chunk_q = np.zeros(nch, dtype=np.int64)
    for ci in range(nch):
        k = (int(sched_t[ci]), int(sched_b[ci]))
        chunk_q[ci] = grp_seen.get(k, 0)
        grp_seen[k] = chunk_q[ci] + 1

    # per-core fill of gather idx / one-hot B (pure 0/1 fp8)
    core_arrays = []
    for c in range(N_CORES):
        pc = per_core[c]
        cnts = counts_all[c]
        starts = np.zeros(T_TILES * N_BANKS + 1, dtype=np.int64)
        np.cumsum(cnts, out=starts[1:])
        gidx = np.zeros((nch, P), dtype=np.int16)
        dstloc = np.full((nch, P), 255, dtype=np.int64)
        for ci in range(nch):
            t, b, q = int(sched_t[ci]), int(sched_b[ci]), int(chunk_q[ci])
            g = t * N_BANKS + b
            s = starts[g] + q * P
            n = min(P, starts[g + 1] - s)
            if n <= 0:
                continue
            sl = slice(s, s + n)
            gidx[ci, :n] = pc["blocal"][sl]
            dstloc[ci, :n] = pc["dstloc"][sl]
        onehot = (dstloc[:, :, None] == np.arange(P)[None, None, :])
        bbig = onehot.astype(ml_dtypes.float8_e4m3fn)
        bbig = np.ascontiguousarray(bbig.transpose(1, 0, 2).reshape(P, nch * P))
        idx16 = gidx.reshape(nch, 8, 16).transpose(2, 0, 1).reshape(16, nch * 8)
        idx128 = np.tile(idx16, (8, 1))
        core_arrays.append({
            "gidx": np.ascontiguousarray(idx128),
            "bbig": bbig,
            "invt": np.ascontiguousarray(pc["inv_tile"].T),  # [P, T_TILES]
        })

    meta = {
        "pos": pos,
        "nch": nch,
        "instrs": instrs,
        "sched_t": sched_t,
        "first_ch": first_ch,
        "last_ch": last_ch,
        "blocks": blocks,
        "has_chunks": (q_tb.sum(axis=1) > 0),
        "hasP": (q_tb[:, :2].sum(axis=1) > 0),
        "hasR": (q_tb[:, 2:].sum(axis=1) > 0),
        "npre_tiles": NPRE_TILES,
    }
    return meta, core_arrays


_GATHER_PATCHED = False


def _relax_gather_elem_assert():
    """dma_gather asserts elem_size_bytes % 256 == 0 (a transpose-mode
    restriction applied unconditionally). The non-transpose ucode handles
    128-byte payloads with a 256-byte row stride (verified on hardware).
    Rebuild the method with the assert relaxed to % 128."""
    global _GATHER_PATCHED
    if _GATHER_PATCHED:
        return
    import inspect
    import re
    import concourse.bass as bassmod

    src = inspect.getsource(bassmod.BassGpSimd.dma_gather)
    src = src.replace(
        "elem_size_bytes > 0 and elem_size_bytes % 256 == 0",
        "elem_size_bytes > 0 and elem_size_bytes % 128 == 0",
    )
    src = re.sub(r"^    def ", "def ", src, count=1, flags=re.M)
    src = "\n".join(l[4:] if l.startswith("    ") else l for l in src.split("\n"))
    ns = dict(bassmod.__dict__)
    exec(compile(src, "patched_dma_gather", "exec"), ns)
    bassmod.BassGpSimd.dma_gather = ns["dma_gather"]
    _GATHER_PATCHED = True


def _build_program(meta):
    import concourse.bass as bass
    import concourse.bacc as bacc
    import concourse.mybir as mybir
    import concourse.tile as tile

    _relax_gather_elem_assert()

    nch = meta["nch"]
    gcols = nch * 8

    nc = bacc.Bacc("TRN2", target_bir_lowering=False, debug=False,
                   num_devices=N_CORES, num_swdge_queues=4)
    f32 = mybir.dt.float32
    bf16 = mybir.dt.bfloat16
    fp8 = mybir.dt.float8e4

    xT_in = nc.dram_tensor("xT", [XD, SHP], bf16, kind="ExternalInput")
    w1t = nc.dram_tensor("w1t", [XD, HID], bf16, kind="ExternalInput")
    b1 = nc.dram_tensor("b1", [HID, 1], f32, kind="ExternalInput")
    wlt = nc.dram_tensor("wlt", [HID, HID], bf16, kind="ExternalInput")
    wra = nc.dram_tensor("wra", [HID + 1, HID], bf16, kind="ExternalInput")
    ident_in = nc.dram_tensor("ident", [P, P], bf16, kind="ExternalInput")
    bbig_in = nc.dram_tensor("bbig", [P, nch * P], fp8, kind="ExternalInput")
    gidx_in = nc.dram_tensor("gidx", [P, gcols], mybir.dt.int16, kind="ExternalInput")
    invt_in = nc.dram_tensor("invt", [P, T_TILES], f32, kind="ExternalInput")

    out_d = nc.dram_tensor("out", [SHP, HID], f32, kind="ExternalOutput")

    HSH = SHP // 2                    # 6272 rows per half per core
    ag_in = [nc.dram_tensor(f"ag_in{h}", [HSH, 2 * HID], bf16)
             for h in range(2)]
    ag_out = [nc.dram_tensor(f"ag_out{h}", [HSH * N_CORES, 2 * HID], bf16,
                             addr_space="Shared")
              for h in range(2)]
    ones_in = nc.dram_tensor("ones", [1, SHP], bf16, kind="ExternalInput")

    with tile.TileContext(nc) as tc:
        with (
            tc.tile_pool(name="const", bufs=1) as cpool,
            tc.tile_pool(name="idx", bufs=1) as ipool,
        ):
            w1t_sb = cpool.tile([P, 4, HID], bf16)
            nc.sync.dma_start(
                out=w1t_sb[:],
                in_=w1t.ap().rearrange("(k p) d -> p k d", p=P),
            )
            b1_sb = cpool.tile([HID, 1], f32)
            nc.sync.dma_start(out=b1_sb[:], in_=b1[:])
            wlt_sb = cpool.tile([HID, HID], bf16)
            nc.scalar.dma_start(out=wlt_sb[:], in_=wlt[:])
            wra_sb = cpool.tile([HID + 1, HID], bf16)
            nc.scalar.dma_start(out=wra_sb[:], in_=wra[:])
            ident_sb = cpool.tile([P, P], bf16)
            nc.scalar.dma_start(out=ident_sb[:], in_=ident_in[:])
            invt_sb = cpool.tile([P, T_TILES], f32)
            nc.scalar.dma_start(out=invt_sb[:], in_=invt_in[:])
            gidx_sb = ipool.tile([P, gcols], mybir.dt.int16)
            nc.scalar.dma_start(out=gidx_sb[:], in_=gidx_in[:])

            # hT_aug [65, SHP] bf16, row 64 = ones (bias fold)
            hT_sb = cpool.tile([HID + 1, SHP], bf16)
            nc.scalar.dma_start(out=hT_sb[HID : HID + 1, :], in_=ones_in[:])

            # ---------------- Phase 1: hT = relu(W1 @ xT + b1) -------------
            with (
                tc.tile_pool(name="xT", bufs=4) as xpool,
                tc.tile_pool(name="p1ps", bufs=4, space="PSUM") as p1ps,
                tc.tile_pool(name="p1tr", bufs=4, space="PSUM") as p1tr,
                tc.tile_pool(name="p1h", bufs=8) as p1h,
            ):
                def emit_transposes(g0, gw, h):
                    ns = gw // P
                    hrow = p1h.tile([P, 4, HID], bf16, tag="hrow", name="hrow")
                    for s in range(ns):
                        tp = p1tr.tile([P, HID], bf16, tag="tp", name="tp",
                                       space="PSUM")
                        nc.tensor.transpose(
                            out=tp[:],
                            in_=hT_sb[:HID, g0 + s * P : g0 + (s + 1) * P],
                            identity=ident_sb[:HID, :HID],
                        )
                        nc.vector.tensor_copy(out=hrow[:, s, :], in_=tp[:])
                    r0 = g0 - h * HSH
                    nc.scalar.dma_start(
                        out=ag_in[h].ap()[r0 : r0 + gw, :HID].rearrange(
                            "(s p) d -> p s d", p=P),
                        in_=hrow[:, :ns, :],
                    )

                XB = 1536
                for h in range(2):
                    pending = None
                    xt = None
                    for g0 in range(h * HSH, (h + 1) * HSH, 512):
                        gw = min(512, (h + 1) * HSH - g0)
                        off = (g0 - h * HSH) % XB
                        if off == 0:
                            xw = min(XB, (h + 1) * HSH - g0)
                            xt = xpool.tile([P, 4, XB], bf16, tag="xt")
                            nc.sync.dma_start(
                                out=xt[:, :, :xw],
                                in_=xT_in.ap()[:, g0 : g0 + xw].rearrange(
                                    "(k p) n -> p k n", p=P),
                            )
                        hps = p1ps.tile([HID, 512], f32, tag="hps", space="PSUM")
                        for k in range(4):
                            nc.tensor.matmul(
                                out=hps[:, :gw],
                                lhsT=w1t_sb[:, k, :],
                                rhs=xt[:, k, off : off + gw],
                                start=(k == 0),
                                stop=(k == 3),
                            )
                        nc.scalar.activation(
                            out=hT_sb[:HID, g0 : g0 + gw], in_=hps[:, :gw],
                            func=mybir.ActivationFunctionType.Relu,
                            bias=b1_sb[:], scale=1.0,
                        )
                        if pending is not None:
                            emit_transposes(*pending)
                        pending = (g0, gw, h)
                    emit_transposes(*pending)
                    # half complete -> fire its AllGather
                    nc.gpsimd.collective_compute(
                        "AllGather",
                        mybir.AluOpType.bypass,
                        replica_groups=[list(range(N_CORES))],
                        ins=[ag_in[h].ap().opt()],
                        outs=[ag_out[h].ap().opt()],
                    )

            # ---------------- Phase 2: gather + aggregate + combine --------
            instrs = meta["instrs"]
            sched_t = meta["sched_t"]
            first_ch = meta["first_ch"]
            last_ch = meta["last_ch"]
            blocks = meta["blocks"]
            has_chunks = meta["has_chunks"]
            hasP = meta["hasP"]
            hasR = meta["hasR"]
            NPRE_TILES = meta["npre_tiles"]
            acc_sb = cpool.tile([HID, NPRE_TILES, P], f32, name="acc_sb")

            with (
                tc.tile_pool(name="msgbf", bufs=24) as mbfpool,
                tc.tile_pool(name="bmat", bufs=24) as bpool,
                tc.tile_pool(name="agg", bufs=1, space="PSUM") as apool,
                tc.tile_pool(name="cps", bufs=1, space="PSUM") as cpspool,
                tc.tile_pool(name="comb", bufs=8) as combpool,
            ):
                ptiles = {}
                blk_done = [False] * len(blocks)

                qn = 0
                for ii, (c0, nch_i, bank) in enumerate(instrs):
                    ni = nch_i * P
                    msgbf = mbfpool.tile([P, MAX_CHUNKS_PER_INSTR * HID], bf16,
                                         tag="msgbf")
                    nc.gpsimd.dma_gather(
                        msgbf[:, : nch_i * HID].rearrange("p (c d) -> p c d", d=HID),
                        ag_out[bank // 2][(bank % 2) * BANK : (bank % 2 + 1) * BANK, :HID],
                        gidx_sb[:, c0 * 8 : c0 * 8 + nch_i * 8],
                        ni, ni, HID,
                        elem_step=2 * HID,
                        queue_num=qn,
                    )
                    qn = (qn + 1) % 4
                    btile = bpool.tile([P, MAX_CHUNKS_PER_INSTR * P], fp8, tag="bt")
                    nc.sync.dma_start(
                        out=btile[:, : nch_i * P],
                        in_=bbig_in[:, c0 * P : (c0 + nch_i) * P],
                    )
                    for k in range(nch_i):
                        ci = c0 + k
                        t = int(sched_t[ci])
                        pp = 0 if (t < NPRE_TILES and bank < 2) else 1
                        if t not in ptiles:
                            ptiles[t] = apool.tile(
                                [HID, P], f32, tag=f"agg{t % 6}",
                                name="aggtile", space="PSUM"
                            )
                        nc.tensor.matmul(
                            out=ptiles[t][:],
                            lhsT=msgbf[:, k * HID : (k + 1) * HID],
                            rhs=btile[:, k * P : (k + 1) * P],
                            start=(ci == first_ch[pp][t]),
                            stop=(ci == last_ch[pp][t]),
                        )
                        if pp == 0 and ci == last_ch[0][t]:
                            nc.vector.tensor_copy(
                                out=acc_sb[:, t, :], in_=ptiles.pop(t)[:])
                    # blocks fully processed? emit their combines
                    nxt = instrs[ii + 1][0] if ii + 1 < len(instrs) else meta["nch"]
                    for blk, (tiles, _, ce) in enumerate(blocks):
                        if blk_done[blk] or nxt < ce:
                            continue
                        blk_done[blk] = True
                        for t in tiles:
                            if t < NPRE_TILES:
                                psum_part = (ptiles.pop(t)
                                             if hasR[t] else None)
                                _combine(nc, mybir, combpool, cpspool,
                                         hT_sb, wlt_sb, wra_sb, invt_sb,
                                         out_d, t, psum_part,
                                         acc_sb if hasP[t] else None)
                            else:
                                psum_part = (ptiles.pop(t)
                                             if has_chunks[t] else None)
                                _combine(nc, mybir, combpool, cpspool,
                                         hT_sb, wlt_sb, wra_sb, invt_sb,
                                         out_d, t, psum_part, None)

    nc.compile()
    return nc


def _combine(nc, mybir, combpool, cpspool, hT_sb, wlt_sb, wra_sb,
             invt_sb, out_d, t, psum_part, acc_sb):
    f32 = mybir.dt.float32
    bf16 = mybir.dt.bfloat16
    cps_b = cpspool.tile([P, HID], f32, tag="cpsb", name="cpsb", space="PSUM")
    nc.tensor.matmul(
        out=cps_b[:],
        lhsT=hT_sb[:, t * P : (t + 1) * P],
        rhs=wra_sb[:],
        start=True, stop=True,
    )
    if psum_part is not None or acc_sb is not None:
        sumT = combpool.tile([HID, P], bf16, tag="sumT", name="sumT")
        if psum_part is not None and acc_sb is not None:
            nc.vector.tensor_tensor(
                out=sumT[:], in0=psum_part[:], in1=acc_sb[:, t, :],
                op=mybir.AluOpType.add,
            )
        elif psum_part is not None:
            nc.vector.tensor_copy(out=sumT[:], in_=psum_part[:])
        else:
            nc.vector.tensor_copy(out=sumT[:], in_=acc_sb[:, t, :])
        cps_a = cpspool.tile([P, HID], f32, tag="cpsa", name="cpsa", space="PSUM")
        nc.tensor.matmul(
            out=cps_a[:], lhsT=sumT[:], rhs=wlt_sb[:],
            start=True, stop=True,
        )
        scaled = combpool.tile([P, HID], f32, tag="scaled", name="scaled")
        nc.vector.tensor_scalar_mul(
            scaled[:], cps_a[:], invt_sb[:, t : t + 1],
        )
        out_sb = combpool.tile([P, HID], f32, tag="outsb", name="outsb")
        nc.vector.tensor_tensor(
            out=out_sb[:], in0=scaled[:], in1=cps_b[:],
            op=mybir.AluOpType.add,
        )
    else:
        out_sb = combpool.tile([P, HID], f32, tag="outsb", name="outsb")
        nc.vector.tensor_copy(out=out_sb[:], in_=cps_b[:])
    nc.scalar.dma_start(out=out_d[t * P : (t + 1) * P, :], in_=out_sb[:])


def kernel(x, edge_index, W1, b1, Wl, bl, Wr):
    from concourse.bass_utils import run_bass_kernel_spmd

    x = np.asarray(x)
    edge_index = np.asarray(edge_index)
    W1 = np.asarray(W1, dtype=np.float32)
    b1v = np.asarray(b1, dtype=np.float32)
    Wl = np.asarray(Wl, dtype=np.float32)
    blv = np.asarray(bl, dtype=np.float32)
    Wr = np.asarray(Wr, dtype=np.float32)

    meta, core_arrays = _prep(edge_index)
    nc = _build_program(meta)

    # host-side transpose + pad + bf16
    xT_np = np.zeros((N_CORES, XD, SHP), dtype=ml_dtypes.bfloat16)
    xb = x.astype(ml_dtypes.bfloat16)
    pos = meta["pos"]
    for c in range(N_CORES):
        xT_np[c, :, pos[c]] = xb[c * SH : (c + 1) * SH]
    w1t_np = np.ascontiguousarray(W1.T).astype(ml_dtypes.bfloat16)
    b1_np = np.ascontiguousarray(b1v[:, None])
    wlt_np = np.ascontiguousarray(Wl.T).astype(ml_dtypes.bfloat16)
    wra_np = np.concatenate([Wr.T, blv[None, :]], axis=0).astype(
        ml_dtypes.bfloat16)
    ident_np = np.eye(P, dtype=np.float32).astype(ml_dtypes.bfloat16)
    ones_np = np.ones((1, SHP), dtype=ml_dtypes.bfloat16)

    in_maps = []
    for c in range(N_CORES):
        ca = core_arrays[c]
        in_maps.append({
            "xT": np.ascontiguousarray(xT_np[c]),
            "w1t": w1t_np,
            "b1": b1_np,
            "wlt": wlt_np,
            "wra": wra_np,
            "ident": ident_np,
            "ones": ones_np,
            "bbig": ca["bbig"],
            "gidx": ca["gidx"],
            "invt": ca["invt"],
        })

    global LAST_EXEC_NS, LAST_RES
    res = run_bass_kernel_spmd(nc, in_maps, list(range(N_CORES)), trace=TRACE)
    LAST_EXEC_NS = res.exec_time_ns
    LAST_RES = res
    out = np.empty((N_NODES, HID), dtype=np.float32)
    for c in range(N_CORES):
        out[c * SH : (c + 1) * SH] = res.results[c]["out"][meta["pos"][c]]
    return out
